# revision 25
# baseline (speedup 1.0000x reference)
"""CPPN forward (12-layer tiny MLP over 4.2M pixels) on 8 TRN2 NeuronCores.

Strategy (pure data parallel, ~3.2x faster than the fp32 baseline):
- Pixels sharded 8 ways; per core 524288 px padded to 208 supertiles (ST).
- One ST = 5 pixel-blocks x 512 px; 5 blocks block-diagonally packed into a
  116-row layout (see row map below; engine partition bases must be
  32-aligned). Group = 4 STs; groups are processed in interleaved pairs
  (A/B software pipeline) and each group is split into two half-chains on
  separate 2-bank PSUM tiles, so DVE/ACT/PE overlap across 4 streams.
- fp16 weights and hidden states: matmuls run at 1 cyc/row (4x faster than
  fp32) and DVE tensor_tensor ops get the 2x packed-16-bit mode. PSUM fp32.
- Per layer per half-group (F=1024):
    2 matmuls [K<=116, M=116, N=512] (fp16) -> Ph (PSUM fp32, 2 banks)
    copyB:  H = Ph + bias; 3 of 4 r-subtiles on DVE tensor_scalar_add,
            one on ACT Identity(+bias), alternating by layer parity for
            engine balance. bias = folded "-1" of the gaussian act
            2*exp(-h^2)-1 (gauss rows x2 in the next weights).
    sin:    s = Sin2pi(P_s/6pi + b') straight from PSUM (one ACT op), then
            ONE DVE square op over rows [64:116] produces s^2 AND the gauss
            squares in one instruction (id rows in between are squared into
            unused temp space - engine cost only depends on free size), then
            H_s3 = s^2*s (DVE). sin(h) = 3s - 4s^3 is carried as TWO
            channels (s at rows 64.., s^3 at rows 32..) and the 3/-4
            combination is folded into the next layer's weight rows - the
            triple angle avoids the Sin2pi spline's +-0.55-turn range limit.
    gauss:  H_g = Exp(-squares) - one full-width ACT op.
- mybir has no Sin2pi enum, so activations are emitted as Sin and the
  serialized BIR JSON is patched Sin->Sin2pi before compilation (the
  exp_and_friends HW table holds {exp, square, identity, copy, sin2pi}).
"""
import sys, types
import numpy as np

sys.path.insert(0, "/opt/trn_rl_repo")

# ---------------------------------------------------------------- constants
N_PIX = 2048 * 2048
D_IN, D_HID, D_OUT = 4, 22, 3
N_HIDDEN = 11
N_CORES = 8
FD = 512                      # pixels per block (= matmul free dim)
BLOCKS = 5                    # blocks per supertile (5*22=110 partitions)
ST_PX = BLOCKS * FD           # 2560 px per supertile
GROUP = 4                     # supertiles per group (PSUM banks)
PX_CORE = N_PIX // N_CORES            # 524288
N_ST = -(-PX_CORE // ST_PX)           # 205
N_GROUP = -(-N_ST // GROUP)           # 52
N_ST_PAD = N_GROUP * GROUP            # 208
PX_PAD = N_ST_PAD * ST_PX             # 532480

ID_CH = list(range(15)) + [19, 20]    # 17 identity channels per block
GA_CH = [15, 16, 17, 18]
SI_CH = [21]
# engine partition bases must be 32-aligned -> layout:
#   rows 0..31   id channels 0..31
#   rows 32..36  sin s^3 (base 32)
#   rows 37..63  id channels 32..58
#   rows 64..68  sin s (base 64)
#   rows 69..94  id channels 59..84
#   row  95      zero pad
#   rows 96..115 gauss (base 96)
# The sin act sin(h)=3s-4s^3 (s=sin(h/3)) is carried as TWO channels (s, s^3);
# the 3/-4 combination is folded into the next layer's weight rows, saving
# two DVE ops per half-group.
ROWS = 116
S30, SIN0, GA0 = 32, 64, 96
TWO_PI = 2.0 * np.pi
SIN_MODE = "triple"           # "direct" | "triple" (see probe results)

# ------------------------------------------------------------- host packing
def _row_of(b, c):
    """partition row of (block b, original channel c) in the ST layout"""
    if c in GA_CH:
        return GA0 + b * 4 + (c - 15)
    if c == 21:
        return SIN0 + b                # the s channel; s^3 lives at S30 + b
    g = b * 17 + ID_CH.index(c)
    if g < 32:
        return g
    if g < 59:
        return 37 + (g - 32)
    return 69 + (g - 59)

SIN_DIV = TWO_PI if SIN_MODE == "direct" else 3 * TWO_PI

def _out_scale(c):
    """scale on weight columns producing channel c's preactivation"""
    return 1.0 / SIN_DIV if c == 21 else 1.0

def _in_scale(c):
    """fold factor on weight rows consuming activation outputs"""
    return 2.0 if c in GA_CH else 1.0

def pack_weights(W_in, W_hidden, W_out):
    """Build the 13 block-diagonal lhsT matrices + bias vectors."""
    W_in, W_hidden, W_out = (np.asarray(W_in, np.float32),
                             np.asarray(W_hidden, np.float32),
                             np.asarray(W_out, np.float32))
    # MM1: x -> layer1 preact. lhsT [20, 116]
    lin = np.zeros((BLOCKS * 4, ROWS), np.float32)
    for b in range(BLOCKS):
        for ci in range(D_IN):
            for co in range(D_HID):
                lin[b * 4 + ci, _row_of(b, co)] = W_in[ci, co] * _out_scale(co)
    # MM2..12: hidden. lhsT [116, 116]
    lh = np.zeros((N_HIDDEN, ROWS, ROWS), np.float32)
    # bt cols 0..11: copyB bias of layer l (col 0 zero) on id+gauss rows.
    bt = np.zeros((ROWS, 12), np.float32)
    # bts cols 0..11: sin-row mod bias b/2pi + SIN_OFF.
    bts = np.zeros((5, 12), np.float32)
    for i in range(N_HIDDEN):
        W = W_hidden[i]
        for b in range(BLOCKS):
            for ci in range(D_HID):
                s = _in_scale(ci)
                ri = _row_of(b, ci)
                for co in range(D_HID):
                    w = W[ci, co] * _out_scale(co)
                    if ci == 21:       # sin act = 3*s - 4*s^3
                        lh[i, ri, _row_of(b, co)] = 3.0 * w
                        lh[i, S30 + b, _row_of(b, co)] = -4.0 * w
                    else:
                        lh[i, ri, _row_of(b, co)] = w * s
        bvec = -W[15:19, :].sum(axis=0)        # folded -1 per output channel
        for b in range(BLOCKS):
            for co in range(D_HID):
                bt[_row_of(b, co), i + 1] = bvec[co]
            bts[b, i + 1] = bvec[21] / SIN_DIV
    # MM13: out. lhsT [116, 15] (+ obias on the packed [111] out layout)
    lo = np.zeros((ROWS, BLOCKS * 3), np.float32)
    for b in range(BLOCKS):
        for ci in range(D_HID):
            s = _in_scale(ci)
            for co in range(D_OUT):
                if ci == 21:
                    lo[_row_of(b, ci), b * 3 + co] = 3.0 * W_out[ci, co]
                    lo[S30 + b, b * 3 + co] = -4.0 * W_out[ci, co]
                else:
                    lo[_row_of(b, ci), b * 3 + co] = W_out[ci, co] * s
    bo = -W_out[15:19, :].sum(axis=0)          # [3]
    obias = np.zeros((111, 1), np.float32)
    for r in range(GROUP):
        for b in range(BLOCKS):
            for co in range(D_OUT):
                obias[32 * r + b * 3 + co, 0] = bo[co]
    return {"w_in": lin.astype(np.float16), "w_hid": lh.astype(np.float16),
            "w_out": lo.astype(np.float16), "bias": bt, "sbias": bts,
            "obias": obias}

def pack_x(x):
    """[N_PIX,4] -> per-core [52, 20, 4, 512] f16 arrays."""
    x = np.asarray(x, np.float32)
    out = []
    for k in range(N_CORES):
        shard = x[k * PX_CORE:(k + 1) * PX_CORE]
        pad = np.zeros((PX_PAD, D_IN), np.float32)
        pad[:PX_CORE] = shard
        a = pad.reshape(N_GROUP, GROUP, BLOCKS, FD, D_IN)
        a = a.transpose(0, 2, 4, 1, 3).reshape(N_GROUP, BLOCKS * D_IN, GROUP, FD)
        out.append(np.ascontiguousarray(a.astype(np.float16)))
    return out

_OUT_ROWS = np.array([[32 * r + b * 3 + co for b in range(BLOCKS) for co in range(D_OUT)]
                      for r in range(GROUP)])  # [4, 15]

def unpack_out(outs):
    """per-core [52, 111, 512] f16 -> [N_PIX, 3] f32"""
    full = np.empty((N_PIX, D_OUT), np.float32)
    for k, od in enumerate(outs):
        g = od.astype(np.float32)[:, _OUT_ROWS.reshape(-1), :]  # [52, 60, 512]
        g = g.reshape(N_GROUP, GROUP, BLOCKS, D_OUT, FD)
        g = g.transpose(0, 1, 2, 4, 3).reshape(PX_PAD, D_OUT)   # [532480, 3]
        full[k * PX_CORE:(k + 1) * PX_CORE] = g[:PX_CORE]
    return full

# ------------------------------------------------------------ device kernel
_CACHE = {}

def _shim_hooks():
    import antenv
    if "antenv.axon_hooks" in sys.modules:
        return
    hooks = types.ModuleType("antenv.axon_hooks")
    hooks._hook = None
    hooks.set_axon_ntff_profile_hook = lambda h: setattr(hooks, "_hook", h)
    hooks.get_axon_ntff_profile_hook = lambda: hooks._hook
    sys.modules["antenv.axon_hooks"] = hooks
    antenv.axon_hooks = hooks
    try:
        from trn_agent_boot.trn_boot import _ntff_profile_via_ctypes
        hooks._hook = _ntff_profile_via_ctypes("/opt/axon/libaxon_pjrt.so")
    except Exception:
        pass

def _build():
    _shim_hooks()
    import concourse.bacc as bacc_mod
    import concourse.mybir as mybir
    import concourse.tile as tile
    from concourse.hw_specs import get_activation_tables as _real_gat

    AFT = mybir.ActivationFunctionType
    ALU = mybir.AluOpType
    ours = {AFT.Square, AFT.Exp, AFT.Identity, AFT.Copy, AFT.Sin, AFT.Relu}

    def _doctored_gat(arch):
        tabs = dict(_real_gat(arch))
        return {n: (set(f) | ours if n == "exp_and_friends" else set(f) - ours)
                for n, f in tabs.items()}

    bacc_mod.get_activation_tables = _doctored_gat

    f32 = mybir.dt.float32
    f16 = mybir.dt.float16
    nc = bacc_mod.Bacc(None, target_bir_lowering=False, debug=False)
    x_d = nc.declare_dram_parameter("x", [N_GROUP, 20, GROUP, FD], f16, isOutput=False)
    win_d = nc.declare_dram_parameter("w_in", [20, ROWS], f16, isOutput=False)
    wh_d = nc.declare_dram_parameter("w_hid", [N_HIDDEN, ROWS, ROWS], f16, isOutput=False)
    wo_d = nc.declare_dram_parameter("w_out", [ROWS, 15], f16, isOutput=False)
    b_d = nc.declare_dram_parameter("bias", [ROWS, 12], f32, isOutput=False)
    bs_d = nc.declare_dram_parameter("sbias", [5, 12], f32, isOutput=False)
    ob_d = nc.declare_dram_parameter("obias", [111, 1], f32, isOutput=False)
    o_d = nc.declare_dram_parameter("out", [N_GROUP, 111, FD], f16, isOutput=True)

    with tile.TileContext(nc) as tc:
        with (tc.tile_pool(name="wpool", bufs=1) as wpool,
              tc.tile_pool(name="xpool", bufs=4) as xpool,
              tc.tile_pool(name="hpool", bufs=2) as hpool,
              tc.tile_pool(name="tpool", bufs=2) as tpool,
              tc.tile_pool(name="opool", bufs=3) as opool,
              tc.tile_pool(name="ppool", bufs=4, space="PSUM") as ppool):
            win = wpool.tile([20, ROWS], f16)
            wh = [wpool.tile([ROWS, ROWS], f16, tag=f"wh{i}", name=f"wh{i}")
                  for i in range(N_HIDDEN)]
            wo = wpool.tile([ROWS, 15], f16)
            bt = wpool.tile([ROWS, 12], f32)
            bs = wpool.tile([5, 12], f32)
            ob = wpool.tile([111, 1], f32)
            nc.sync.dma_start(out=win[:], in_=win_d[:])
            for i in range(N_HIDDEN):
                nc.sync.dma_start(out=wh[i][:], in_=wh_d[i])
            nc.sync.dma_start(out=wo[:], in_=wo_d[:])
            nc.sync.dma_start(out=bt[:], in_=b_d[:])
            nc.sync.dma_start(out=bs[:], in_=bs_d[:])
            nc.sync.dma_start(out=ob[:], in_=ob_d[:])

            HC = GROUP // 2                   # copyB split point (DVE|ACT)
            for pr in range(N_GROUP // 2):
                gpair = (2 * pr, 2 * pr + 1)
                xg = {}
                for par, g in enumerate(gpair):
                    xg[par] = xpool.tile([20, GROUP, FD], f16, tag="xg",
                                         name=f"xg{par}")
                    nc.sync.dma_start(out=xg[par][:], in_=x_d[g])
                hprev = {0: None, 1: None}
                for mm in range(1, 13):       # 12 hidden matmul rounds
                    for par in range(2):
                        H = hpool.tile([ROWS, GROUP, FD], f16, tag=f"H{par}")
                        # one squares tile spanning [64:116]: a single TT
                        # square covers the s rows AND the gauss rows (id rows
                        # in between get squared into unused temp space).
                        SQ = tpool.tile([ROWS, GROUP, FD], f16, tag=f"sq{par}")
                        # half-group granular chains on split PSUM tiles: each
                        # half has its own 2-bank P tile, so elementwise work
                        # starts after 2 matmuls and P banks recycle per-half.
                        for hh in range(2):
                            rs = slice(2 * hh, 2 * hh + 2)
                            Ph = ppool.tile([ROWS, 2, FD], f32, tag="pm",
                                            name=f"P{par}{hh}")
                            for r in range(2):
                                if mm == 1:
                                    nc.tensor.matmul(Ph[:, r, :], win[:],
                                                     xg[par][:, 2 * hh + r, :],
                                                     start=True, stop=True)
                                else:
                                    nc.tensor.matmul(Ph[:, r, :], wh[mm - 2][:],
                                                     hprev[par][:, 2 * hh + r, :],
                                                     start=True, stop=True)
                            # copyB: H = P + bias. DVE takes r0,r1 and one
                            # of r2/r3; the other goes to ACT, alternating by
                            # layer parity for engine balance.
                            if hh == 0:
                                nc.vector.tensor_scalar_add(
                                    H[:, rs, :], Ph[:, :, :], bt[:, mm - 1:mm])
                            else:
                                dv, av = (0, 1) if mm % 2 == 0 else (1, 0)
                                nc.vector.tensor_scalar_add(
                                    H[:, 2 + dv:3 + dv, :], Ph[:, dv:dv + 1, :],
                                    bt[:, mm - 1:mm])
                                nc.scalar.activation(
                                    H[:, 2 + av:3 + av, :], Ph[:, av:av + 1, :],
                                    AFT.Identity,
                                    bias=bt[:, mm - 1:mm], scale=1.0)
                            # s = Sin2pi(P_s/6pi + b') straight from PSUM,
                            # written into H's s rows.
                            nc.scalar.activation(H[SIN0:SIN0 + 5, rs, :],
                                                 Ph[SIN0:SIN0 + 5, :, :],
                                                 AFT.Sin, bias=bs[:, mm - 1:mm],
                                                 scale=1.0)
                            # ONE square op per half covers s rows and gauss
                            # rows (id rows in between are squared into unused
                            # temp space - cost depends only on free size).
                            nc.vector.tensor_tensor(SQ[SIN0:, rs, :],
                                                    H[SIN0:, rs, :],
                                                    H[SIN0:, rs, :],
                                                    op=ALU.mult)
                            # sin cube channel: H_s3 = s^2 * s
                            nc.vector.tensor_tensor(H[S30:S30 + 5, rs, :],
                                                    SQ[SIN0:SIN0 + 5, rs, :],
                                                    H[SIN0:SIN0 + 5, rs, :],
                                                    op=ALU.mult)
                        # gauss: one full-width exp from the squares tile
                        nc.scalar.activation(H[GA0:GA0 + 20, :, :],
                                             SQ[GA0:GA0 + 20, :, :],
                                             AFT.Exp, bias=0.0, scale=-1.0)
                        hprev[par] = H
                for par, g in enumerate(gpair):   # output round
                    O = ppool.tile([111, FD], f32, tag="pm")
                    for r in range(GROUP):
                        nc.tensor.matmul(O[32 * r:32 * r + 15, :], wo[:],
                                         hprev[par][:, r, :], start=True,
                                         stop=True, tile_position=(0, 32 * r))
                    ot = opool.tile([111, FD], f16, tag="ot")
                    nc.vector.tensor_scalar_add(ot[:], O[:], ob[:])
                    nc.sync.dma_start(out=o_d[g], in_=ot[:])
    nc.compile()

    _orig = nc.to_json_bytes
    nc.to_json_bytes = lambda: _orig().replace(b'"func":"Sin"', b'"func":"Sin2pi"')
    return nc

def _get_nc():
    if "nc" not in _CACHE:
        _CACHE["nc"] = _build()
    return _CACHE["nc"]

def make_in_maps(x_cores, w):
    return [{"x": x_cores[k], "w_in": w["w_in"], "w_hid": w["w_hid"],
             "w_out": w["w_out"], "bias": w["bias"], "sbias": w["sbias"],
             "obias": w["obias"]} for k in range(N_CORES)]

def run_device(x_cores, w):
    from concourse.bass_utils import run_bass_kernel_spmd
    nc = _get_nc()
    res = run_bass_kernel_spmd(nc, make_in_maps(x_cores, w),
                               list(range(N_CORES)), trace=False)
    return [res.results[k]["out"] for k in range(N_CORES)]

def kernel(x, W_in, W_hidden, W_out):
    w = pack_weights(W_in, W_hidden, W_out)
    x_cores = pack_x(x)
    outs = run_device(x_cores, w)
    return unpack_out(outs)
